# revision 1
# baseline (speedup 1.0000x reference)
# Trainium2 Bass kernel for nn_LAB_42906723287350.
#
#   probs = softmax(choice_parameters, axis=0); s = x @ probs
#   out = mix0*multilinear(sigmoid(lut); s) + mix1*clip(s0+s1+s2-2,0,1)
#         + mix2*(s0+s1+s2>=2)
#
# Data parallel over 8 cores (R rows each).  Per core:
#   x2 tiles [128, 128] (2 rows/partition, 512B descriptors) -> PE transpose
#   -> PSUM->SBUF copy (cast bf16) -> matmul vs stationary P2 [128,8]
#   (P2[t*64+c, 2j+t'] = probs[c,j]*[t==t']) -> s^T -> PE transpose back
#   -> 16-coeff multilinear Horner poly + add path on ACT/DVE -> store.
import numpy as np
import ml_dtypes

import concourse.bass as bass
import concourse.mybir as mybir
import concourse.tile as tile
from concourse import bacc
from concourse.bass_utils import run_bass_kernel_spmd
from concourse.masks import make_identity

N_CORES = 8
B_FULL = 1048576
CIN = 64
F32 = mybir.dt.float32
MM_DT = mybir.dt.bfloat16
SB_ROWS = 4096      # rows per superblock (one sT psum tile: 4 groups x 1024)
PB_SBS = 8          # superblocks per poly batch (32768 rows)
ALU = mybir.AluOpType
AF = mybir.ActivationFunctionType


def build_nc(R, mm_dtype=MM_DT):
    assert R % (PB_SBS * SB_ROWS) == 0
    n_sb = R // SB_ROWS

    nc = bacc.Bacc()
    x_d = nc.dram_tensor("x", [R, CIN], F32, kind="ExternalInput")
    pab_d = nc.dram_tensor("pab", [128, 8], mm_dtype, kind="ExternalInput")
    coef_d = nc.dram_tensor("coef", [128, 18], F32, kind="ExternalInput")
    out_d = nc.dram_tensor("out", [R, 1], F32, kind="ExternalOutput")

    # x row b = 4096*sb + 1024*gg + 256*nb + 2*p + t
    x2v = x_d[:].rearrange("(n p t) c -> n p (t c)", p=128, t=2)
    # out row b = 32768*batch + 4096*b4 + 1024*gg + 256*ch + 2*p + t
    outv = out_d[:].rearrange(
        "(sb gg ch p t) one -> sb gg p ch (t one)", gg=4, ch=4, p=128, t=2
    )

    with tile.TileContext(nc) as tc:
        with (
            tc.tile_pool(name="const", bufs=1) as cpool,
            tc.tile_pool(name="xin", bufs=12) as xpool,
            tc.tile_pool(name="work", bufs=3) as pool,
            tc.tile_pool(name="tmp", bufs=2) as tpool,
            tc.tile_pool(name="psum", bufs=2, space="PSUM") as ppool,
            tc.tile_pool(name="psumx", bufs=4, space="PSUM") as ppoolx,
        ):
            ident = cpool.tile([128, 128], F32)
            make_identity(nc, ident[:])
            pab_sb = cpool.tile([128, 8], mm_dtype)
            nc.sync.dma_start(out=pab_sb[:], in_=pab_d[:])
            coef_sb = cpool.tile([128, 18], F32)
            nc.sync.dma_start(out=coef_sb[:], in_=coef_d[:])

            def poly_and_store(s_nat, sb):
                # s_nat [128, 512*PB_SBS]; free = bc*128 + gg*32 + j*2 + t
                # (bc = b4*4+ch merged; j*2+t < 8 live, rest dead)
                BC = PB_SBS * 4
                sv = s_nat[:].rearrange(
                    "p (bc gg j t) -> p bc gg j t", bc=BC, gg=4, j=16, t=2
                )
                s = [sv[:, :, :, jj, :] for jj in range(4)]

                def tmp(tag):
                    tl = tpool.tile([128, BC * 8], F32, tag=tag)
                    return tl[:].rearrange(
                        "p (bc gg t) -> p bc gg t", bc=BC, gg=4, t=2
                    )

                Lf = []
                for i in range(8):
                    v = tmp(f"leaf{i}")
                    nc.scalar.activation(
                        v, s[0], AF.Identity,
                        bias=coef_sb[:, i : i + 1],
                        scale=coef_sb[:, 8 + i : 9 + i],
                    )
                    Lf.append(v)
                G = []
                for k in range(4):
                    v = tmp(f"gm{k}")
                    nc.vector.tensor_mul(v, s[1], Lf[2 * k + 1])
                    nc.vector.tensor_add(v, v, Lf[2 * k])
                    G.append(v)
                H = []
                for m in range(2):
                    v = tmp(f"hm{m}")
                    nc.vector.tensor_mul(v, s[2], G[2 * m + 1])
                    nc.vector.tensor_add(v, v, G[2 * m])
                    H.append(v)
                vl = tmp("lut")
                nc.vector.tensor_mul(vl, s[3], H[1])
                nc.vector.tensor_add(vl, vl, H[0])
                va = tmp("adds")
                nc.vector.tensor_add(va, s[0], s[1])
                nc.vector.tensor_add(va, va, s[2])
                vr = tmp("relu")
                nc.vector.tensor_scalar(vr, va, -2.0, 0.0, op0=ALU.add, op1=ALU.max)
                nc.vector.tensor_scalar(
                    vr, vr, 1.0, coef_sb[:, 16:17], op0=ALU.min, op1=ALU.mult
                )
                vq = tmp("step")
                nc.vector.tensor_scalar(
                    vq, va, 2.0, coef_sb[:, 17:18], op0=ALU.is_ge, op1=ALU.mult
                )
                ot = tpool.tile([128, BC * 8], F32, tag="outsb")
                vo = ot[:].rearrange("p (bc gg t) -> p bc gg t", bc=BC, gg=4, t=2)
                nc.vector.tensor_add(vo, vl, vr)
                nc.vector.tensor_add(vo, vo, vq)
                for b4 in range(PB_SBS):
                    for gg in range(4):
                        nc.sync.dma_start(
                            out=outv[sb - PB_SBS + 1 + b4, gg],
                            in_=vo[:, 4 * b4 : 4 * b4 + 4, gg],
                        )

            s_nat = None
            for sb in range(n_sb):
                sT_ps = ppool.tile([128, 512], F32, tag="sT")
                for gg in range(4):
                    xt_ps = ppoolx.tile([128, 512], F32, tag="xt")
                    for nb in range(4):
                        n = sb * 16 + gg * 4 + nb
                        xt = xpool.tile([128, 128], F32, tag="x2")
                        nc.sync.dma_start(out=xt[:], in_=x2v[n])
                        nc.tensor.transpose(
                            xt_ps[:, 128 * nb : 128 * (nb + 1)], xt[:], ident[:]
                        )
                    xt_sb = pool.tile([128, 512], mm_dtype, tag="xt_sb")
                    if gg % 2 == 0:
                        nc.scalar.copy(out=xt_sb[:], in_=xt_ps[:])
                    else:
                        nc.vector.tensor_copy(out=xt_sb[:], in_=xt_ps[:])
                    nc.tensor.matmul(
                        sT_ps[32 * gg : 32 * gg + 8, :], lhsT=pab_sb[:],
                        rhs=xt_sb[:], start=True, stop=True,
                        tile_position=(0, 32 * gg),
                    )
                sT_sb = pool.tile([128, 512], F32, tag="sT_sb")
                nc.scalar.copy(out=sT_sb[:], in_=sT_ps[:])
                sN_ps = ppool.tile([128, 512], F32, tag="sN")
                for ch in range(4):
                    nc.tensor.transpose(
                        sN_ps[:, 128 * ch : 128 * (ch + 1)],
                        sT_sb[:, 128 * ch : 128 * (ch + 1)], ident[:],
                    )
                if sb % PB_SBS == 0:
                    s_nat = pool.tile([128, 512 * PB_SBS], F32, tag="snat")
                if sb % 2 == 0:
                    nc.scalar.copy(
                        out=s_nat[:, 512 * (sb % PB_SBS) : 512 * (sb % PB_SBS + 1)],
                        in_=sN_ps[:],
                    )
                else:
                    nc.vector.tensor_copy(
                        out=s_nat[:, 512 * (sb % PB_SBS) : 512 * (sb % PB_SBS + 1)],
                        in_=sN_ps[:],
                    )
                if sb % PB_SBS == PB_SBS - 1:
                    poly_and_store(s_nat, sb)
    nc.compile()
    return nc


def host_prep(choice_parameters, lut, lut_vs_add_choice_parameters, mm_np=ml_dtypes.bfloat16):
    cp = np.asarray(choice_parameters, dtype=np.float64)
    e = np.exp(cp - cp.max(axis=0, keepdims=True))
    probs = e / e.sum(axis=0, keepdims=True)  # [64,4]
    L = 1.0 / (1.0 + np.exp(-np.asarray(lut, dtype=np.float64)))
    m = np.asarray(lut_vs_add_choice_parameters, dtype=np.float64)
    em = np.exp(m - m.max())
    mix = em / em.sum()

    c = np.zeros(16)
    for S in range(16):
        v = L
        for ax in range(4):
            vec = np.array([1.0, -1.0]) if (S >> ax) & 1 else np.array([0.0, 1.0])
            v = np.tensordot(v, vec, axes=([0], [0]))
        c[S] = float(v) * mix[0]

    coef_row = np.zeros(18)
    for idx in range(8):
        coef_row[idx] = c[idx << 1]
        coef_row[8 + idx] = c[(idx << 1) | 1]
    coef_row[16] = mix[1]
    coef_row[17] = mix[2]
    coef = np.tile(coef_row.astype(np.float32)[None], (128, 1))

    # P2[q=(t,c), m=(j,t')] = probs[c, j] * [t==t']
    pab = np.zeros((128, 8), np.float64)
    for t in range(2):
        for cc in range(64):
            for j in range(4):
                pab[t * 64 + cc, j * 2 + t] = probs[cc, j]
    pab = pab.astype(mm_np)
    return pab, coef


_NC_CACHE = {}


def _get_nc(R):
    if R not in _NC_CACHE:
        _NC_CACHE[R] = build_nc(R)
    return _NC_CACHE[R]


def run_on_hw(x, choice_parameters, lut, lut_vs_add_choice_parameters, **kw):
    x = np.ascontiguousarray(np.asarray(x, dtype=np.float32))
    R = x.shape[0] // N_CORES
    nc = _get_nc(R)
    pab, coef = host_prep(choice_parameters, lut, lut_vs_add_choice_parameters)
    in_maps = [
        {"x": np.ascontiguousarray(x[i * R : (i + 1) * R]), "pab": pab, "coef": coef}
        for i in range(N_CORES)
    ]
    res = run_bass_kernel_spmd(nc, in_maps, list(range(N_CORES)), **kw)
    out = np.concatenate([r["out"] for r in res.results], axis=0)
    return out, res


def kernel(x, choice_parameters, lut, lut_vs_add_choice_parameters):
    out, _ = run_on_hw(x, choice_parameters, lut, lut_vs_add_choice_parameters)
    return out



# revision 3
# speedup vs baseline: 3.2533x; 3.2533x over previous
# Trainium2 Bass kernel for nn_LAB_42906723287350.
#
#   probs = softmax(choice_parameters, axis=0); s = x @ probs
#   out = mix0*multilinear(sigmoid(lut); s) + mix1*clip(s0+s1+s2-2,0,1)
#         + mix2*(s0+s1+s2>=2)
#
# Data parallel over 8 cores (R=131072 rows each).  Row mapping per core:
#   row = 1024*nd + 8*p + t   (nd = tile, p = partition, t in [0,8))
# Per tile nd: DMA x [128, (t,c)=512] f32 (2KB/partition descriptors)
#   -> 4 PE transposes -> xT [(t2,c), p] psum -> copy/cast bf16
#   -> 4 accumulating matmuls with xT as STATIONARY and P8 [128,32] moving
#      -> s in NATURAL layout [128 p, (j,t)] (no back-transpose needed).
# Per group g (16 tiles): copy s psum->SBUF, 16-coeff Horner poly + add
#   path -> vo2 [128, (t,nd)] -> one PE transpose -> [(t,nd), p] psum
#   -> 8 strided copies into S [128 (g,nd), (p,t)] -> contiguous 64KB
#   output DMA per group (4KB/partition descriptors).
import numpy as np
import ml_dtypes

import concourse.bass as bass
import concourse.mybir as mybir
import concourse.tile as tile
from concourse import bacc
from concourse.bass_utils import run_bass_kernel_spmd
from concourse.masks import make_identity

N_CORES = 8
B_FULL = 1048576
CIN = 64
F32 = mybir.dt.float32
MM_DT = mybir.dt.bfloat16
ALU = mybir.AluOpType
AF = mybir.ActivationFunctionType

T_IL = 8          # rows interleaved per partition (input descriptor = 256B*T_IL)
ROWS_TILE = 128 * T_IL          # 1024 rows per x-tile
TILES_GRP = 32                  # tiles per poly group (32-aligned partition slices)
ROWS_GRP = ROWS_TILE * TILES_GRP  # 32768 rows


def build_nc(R, mm_dtype=MM_DT):
    n_tiles = R // ROWS_TILE
    n_grp = n_tiles // TILES_GRP
    assert R == n_grp * ROWS_GRP and n_grp <= 4

    nc = bacc.Bacc()
    x_d = nc.dram_tensor("x", [R, CIN], F32, kind="ExternalInput")
    p8_d = nc.dram_tensor("p8", [128, 128], mm_dtype, kind="ExternalInput")
    coef_d = nc.dram_tensor("coef", [128, 18], F32, kind="ExternalInput")
    out_d = nc.dram_tensor("out", [R, 1], F32, kind="ExternalOutput")

    # x row = 1024*nd + 8*p + t ; tile view [nd, p, (t c)]
    x2v = x_d[:].rearrange("(nd p t) c -> nd p (t c)", p=128, t=T_IL)
    # out row = 1024*(16g+k) + 8*p + t ; [(g k), (p t)]
    outv = out_d[:].rearrange(
        "(gk p t) one -> gk (p t one)", gk=n_grp * TILES_GRP, p=128, t=T_IL
    )

    with tile.TileContext(nc) as tc:
        with (
            tc.tile_pool(name="const", bufs=1) as cpool,
            tc.tile_pool(name="xin", bufs=6) as xpool,
            tc.tile_pool(name="xtsb", bufs=4) as xsbpool,
            tc.tile_pool(name="snat", bufs=2) as snpool,
            tc.tile_pool(name="tmp", bufs=2) as tpool,
            tc.tile_pool(name="outsb", bufs=1) as spool,
            tc.tile_pool(name="psxt", bufs=2, space="PSUM") as ppxt,
            tc.tile_pool(name="pssn", bufs=2, space="PSUM") as ppsn,
            tc.tile_pool(name="psvo", bufs=2, space="PSUM") as ppvo,
        ):
            ident = cpool.tile([128, 128], F32)
            make_identity(nc, ident[:])
            p8_sb = cpool.tile([128, 128], mm_dtype)
            nc.sync.dma_start(out=p8_sb[:], in_=p8_d[:])
            coef_sb = cpool.tile([128, 18], F32)
            nc.sync.dma_start(out=coef_sb[:], in_=coef_d[:])

            S = spool.tile([128, 128 * T_IL], F32)  # [ (g k), (p t) ]
            Sv = S[:].rearrange("q (p t) -> q p t", p=128, t=T_IL)

            def poly_group(sn_sb, g):
                # sn_sb [128, (k j t)] f32; s_j(row) at [p, k, j, t]
                sv = sn_sb[:].rearrange(
                    "p (k j t) -> p k j t", k=TILES_GRP, j=4, t=T_IL
                )
                s = [sv[:, :, jj, :] for jj in range(4)]

                def tmp(tag):
                    tl = tpool.tile([128, TILES_GRP * T_IL], F32, tag=tag)
                    return tl, tl[:].rearrange(
                        "p (k t) -> p k t", k=TILES_GRP, t=T_IL
                    )

                Lf = []
                for i in range(8):
                    _, v = tmp(f"leaf{i}")
                    nc.scalar.activation(
                        v, s[0], AF.Identity,
                        bias=coef_sb[:, i : i + 1],
                        scale=coef_sb[:, 8 + i : 9 + i],
                    )
                    Lf.append(v)
                G = []
                for k in range(4):
                    _, v = tmp(f"gm{k}")
                    nc.vector.tensor_mul(v, s[1], Lf[2 * k + 1])
                    nc.vector.tensor_add(v, v, Lf[2 * k])
                    G.append(v)
                H = []
                for m in range(2):
                    _, v = tmp(f"hm{m}")
                    nc.vector.tensor_mul(v, s[2], G[2 * m + 1])
                    nc.vector.tensor_add(v, v, G[2 * m])
                    H.append(v)
                _, vl = tmp("lut")
                nc.vector.tensor_mul(vl, s[3], H[1])
                nc.vector.tensor_add(vl, vl, H[0])
                _, va = tmp("adds")
                nc.vector.tensor_add(va, s[0], s[1])
                nc.vector.tensor_add(va, va, s[2])
                _, vr = tmp("relu")
                nc.vector.tensor_scalar(vr, va, -2.0, 0.0, op0=ALU.add, op1=ALU.max)
                nc.vector.tensor_scalar(
                    vr, vr, 1.0, coef_sb[:, 16:17], op0=ALU.min, op1=ALU.mult
                )
                _, vq = tmp("step")
                nc.vector.tensor_scalar(
                    vq, va, 2.0, coef_sb[:, 17:18], op0=ALU.is_ge, op1=ALU.mult
                )
                # vo2 [128, (t k)] so partitions after transpose group by t
                vo2 = tpool.tile([128, TILES_GRP * T_IL], F32, tag="vo2")
                vo2v = vo2[:].rearrange("p (t k) -> p k t", t=T_IL, k=TILES_GRP)
                nc.vector.tensor_add(vo2v, vl, vr)
                nc.vector.tensor_add(vo2v, vo2v, vq)
                # two transposes -> voT[:, 128b:...] = [(tl k), p] for t = 4b+tl
                voT = ppvo.tile([128, 256], F32, tag="voT")
                nc.tensor.transpose(voT[:, 0:128], vo2[:, 0:128], ident[:])
                nc.tensor.transpose(voT[:, 128:256], vo2[:, 128:256], ident[:])
                for t in range(T_IL):
                    b, tl = divmod(t, 4)
                    nc.scalar.copy(
                        out=Sv[TILES_GRP * g : TILES_GRP * (g + 1), :, t],
                        in_=voT[32 * tl : 32 * (tl + 1), 128 * b : 128 * (b + 1)],
                    )
                nc.sync.dma_start(
                    out=outv[TILES_GRP * g : TILES_GRP * (g + 1)],
                    in_=S[TILES_GRP * g : TILES_GRP * (g + 1), :],
                )

            for g in range(n_grp):
                sn_sb = snpool.tile([128, 32 * TILES_GRP], F32, tag="snsb")
                for half in range(2):
                    sn_ps = ppsn.tile([128, 512], F32, tag="sn")
                    for kk in range(16):
                        k = 16 * half + kk
                        nd = g * TILES_GRP + k
                        xt = xpool.tile([128, 128 * 4], F32, tag="x")
                        nc.sync.dma_start(out=xt[:], in_=x2v[nd])
                        xT_ps = ppxt.tile([128, 512], F32, tag="xT")
                        for h in range(4):
                            nc.tensor.transpose(
                                xT_ps[:, 128 * h : 128 * (h + 1)],
                                xt[:, 128 * h : 128 * (h + 1)],
                                ident[:],
                            )
                        xT_sb = xsbpool.tile([128, 512], mm_dtype, tag="xTsb")
                        if nd % 2 == 0:
                            nc.scalar.copy(out=xT_sb[:], in_=xT_ps[:])
                        else:
                            nc.vector.tensor_copy(out=xT_sb[:], in_=xT_ps[:])
                        for h in range(4):
                            nc.tensor.matmul(
                                sn_ps[:, 32 * kk : 32 * (kk + 1)],
                                lhsT=xT_sb[:, 128 * h : 128 * (h + 1)],
                                rhs=p8_sb[:, 32 * h : 32 * (h + 1)],
                                start=(h == 0),
                                stop=(h == 3),
                            )
                    if half == 0:
                        nc.vector.tensor_copy(out=sn_sb[:, 0:512], in_=sn_ps[:])
                    else:
                        nc.scalar.copy(out=sn_sb[:, 512:1024], in_=sn_ps[:])
                poly_group(sn_sb, g)
    nc.compile()
    return nc


def host_prep(choice_parameters, lut, lut_vs_add_choice_parameters, mm_np=ml_dtypes.bfloat16):
    cp = np.asarray(choice_parameters, dtype=np.float64)
    e = np.exp(cp - cp.max(axis=0, keepdims=True))
    probs = e / e.sum(axis=0, keepdims=True)  # [64,4]
    L = 1.0 / (1.0 + np.exp(-np.asarray(lut, dtype=np.float64)))
    m = np.asarray(lut_vs_add_choice_parameters, dtype=np.float64)
    em = np.exp(m - m.max())
    mix = em / em.sum()

    c = np.zeros(16)
    for Sm in range(16):
        v = L
        for ax in range(4):
            vec = np.array([1.0, -1.0]) if (Sm >> ax) & 1 else np.array([0.0, 1.0])
            v = np.tensordot(v, vec, axes=([0], [0]))
        c[Sm] = float(v) * mix[0]

    coef_row = np.zeros(18)
    for idx in range(8):
        coef_row[idx] = c[idx << 1]
        coef_row[8 + idx] = c[(idx << 1) | 1]
    coef_row[16] = mix[1]
    coef_row[17] = mix[2]
    coef = np.tile(coef_row.astype(np.float32)[None], (128, 1))

    # P8[h][ (t2,c), (j,t) ] = probs[c,j] * [t == 2h+t2]; packed as
    # p8[t2*64+c, 32*h + 8*j + t]
    p8 = np.zeros((128, 128), np.float64)
    for h in range(4):
        for t2 in range(2):
            for cc in range(64):
                for j in range(4):
                    p8[t2 * 64 + cc, 32 * h + 8 * j + (2 * h + t2)] = probs[cc, j]
    p8 = p8.astype(mm_np)
    return p8, coef


_NC_CACHE = {}


def _get_nc(R):
    if R not in _NC_CACHE:
        _NC_CACHE[R] = build_nc(R)
    return _NC_CACHE[R]


def run_on_hw(x, choice_parameters, lut, lut_vs_add_choice_parameters, **kw):
    x = np.ascontiguousarray(np.asarray(x, dtype=np.float32))
    R = x.shape[0] // N_CORES
    nc = _get_nc(R)
    p8, coef = host_prep(choice_parameters, lut, lut_vs_add_choice_parameters)
    in_maps = [
        {"x": np.ascontiguousarray(x[i * R : (i + 1) * R]), "p8": p8, "coef": coef}
        for i in range(N_CORES)
    ]
    res = run_bass_kernel_spmd(nc, in_maps, list(range(N_CORES)), **kw)
    out = np.concatenate([r["out"] for r in res.results], axis=0)
    return out, res


def kernel(x, choice_parameters, lut, lut_vs_add_choice_parameters):
    out, _ = run_on_hw(x, choice_parameters, lut, lut_vs_add_choice_parameters)
    return out


# revision 4
# speedup vs baseline: 3.6403x; 1.1189x over previous
# Trainium2 Bass kernel for nn_LAB_42906723287350.
#
#   probs = softmax(choice_parameters, axis=0); s = x @ probs
#   out = mix0*multilinear(sigmoid(lut); s) + mix1*clip(s0+s1+s2-2,0,1)
#         + mix2*(s0+s1+s2>=2)
#
# Data parallel over 8 cores (R=131072 rows each).  Row mapping per core:
#   row = 1024*nd + 8*p + t   (nd = tile, p = partition, t in [0,8))
# Per tile nd: DMA x [128, (t,c)=512] f32 (2KB/partition descriptors)
#   -> 4 PE transposes -> xT [(t2,c), p] psum -> copy/cast bf16
#   -> 4 accumulating matmuls with xT as STATIONARY and P8 [128,32] moving
#      -> s in NATURAL layout [128 p, (j,t)] (no back-transpose needed).
# Per group g (16 tiles): copy s psum->SBUF, 16-coeff Horner poly + add
#   path -> vo2 [128, (t,nd)] -> one PE transpose -> [(t,nd), p] psum
#   -> 8 strided copies into S [128 (g,nd), (p,t)] -> contiguous 64KB
#   output DMA per group (4KB/partition descriptors).
import numpy as np
import ml_dtypes

import concourse.bass as bass
import concourse.mybir as mybir
import concourse.tile as tile
from concourse import bacc
from concourse.bass_utils import run_bass_kernel_spmd
from concourse.masks import make_identity

N_CORES = 8
B_FULL = 1048576
CIN = 64
F32 = mybir.dt.float32
MM_DT = mybir.dt.bfloat16
ALU = mybir.AluOpType
AF = mybir.ActivationFunctionType

T_IL = 8          # rows interleaved per partition (input descriptor = 256B*T_IL)
ROWS_TILE = 128 * T_IL          # 1024 rows per x-tile
TILES_GRP = 32                  # tiles per poly group (32-aligned partition slices)
ROWS_GRP = ROWS_TILE * TILES_GRP  # 32768 rows


def build_nc(R, mm_dtype=MM_DT):
    n_tiles = R // ROWS_TILE
    n_grp = n_tiles // TILES_GRP
    assert R == n_grp * ROWS_GRP and n_grp <= 4

    nc = bacc.Bacc()
    x_d = nc.dram_tensor("x", [R, CIN], F32, kind="ExternalInput")
    p8_d = nc.dram_tensor("p8", [128, 128], mm_dtype, kind="ExternalInput")
    coef_d = nc.dram_tensor("coef", [128, 18], F32, kind="ExternalInput")
    out_d = nc.dram_tensor("out", [R, 1], F32, kind="ExternalOutput")

    # x row = 1024*nd + 8*p + t ; tile view [nd, p, (t c)]
    x2v = x_d[:].rearrange("(nd p t) c -> nd p (t c)", p=128, t=T_IL)
    # out row = 1024*(16g+k) + 8*p + t ; [(g k), (p t)]
    outv = out_d[:].rearrange(
        "(gk p t) one -> gk (p t one)", gk=n_grp * TILES_GRP, p=128, t=T_IL
    )

    with tile.TileContext(nc) as tc:
        with (
            tc.tile_pool(name="const", bufs=1) as cpool,
            tc.tile_pool(name="xin", bufs=6) as xpool,
            tc.tile_pool(name="xtsb", bufs=4) as xsbpool,
            tc.tile_pool(name="snat", bufs=2) as snpool,
            tc.tile_pool(name="tmp", bufs=2) as tpool,
            tc.tile_pool(name="outsb", bufs=1) as spool,
            tc.tile_pool(name="psxt", bufs=2, space="PSUM") as ppxt,
            tc.tile_pool(name="pssn", bufs=2, space="PSUM") as ppsn,
            tc.tile_pool(name="psvo", bufs=2, space="PSUM") as ppvo,
        ):
            ident = cpool.tile([128, 128], F32)
            make_identity(nc, ident[:])
            identb = cpool.tile([128, 128], mm_dtype)
            make_identity(nc, identb[:])
            p8_sb = cpool.tile([128, 128], mm_dtype)
            nc.sync.dma_start(out=p8_sb[:], in_=p8_d[:])
            coef_sb = cpool.tile([128, 18], F32)
            nc.sync.dma_start(out=coef_sb[:], in_=coef_d[:])

            S = spool.tile([128, 128 * T_IL], F32)  # [ (g k), (p t) ]
            Sv = S[:].rearrange("q (p t) -> q p t", p=128, t=T_IL)

            def poly_group(sn_sb, g):
                # sn_sb [128, (k j t)] f32; s_j(row) at [p, k, j, t]
                sv = sn_sb[:].rearrange(
                    "p (k j t) -> p k j t", k=TILES_GRP, j=4, t=T_IL
                )
                s = [sv[:, :, jj, :] for jj in range(4)]

                def tmp(tag):
                    tl = tpool.tile([128, TILES_GRP * T_IL], F32, tag=tag)
                    return tl, tl[:].rearrange(
                        "p (k t) -> p k t", k=TILES_GRP, t=T_IL
                    )

                Lf = []
                for i in range(8):
                    _, v = tmp(f"leaf{i}")
                    nc.scalar.activation(
                        v, s[0], AF.Identity,
                        bias=coef_sb[:, i : i + 1],
                        scale=coef_sb[:, 8 + i : 9 + i],
                    )
                    Lf.append(v)
                G = []
                for k in range(4):
                    _, v = tmp(f"gm{k}")
                    nc.vector.tensor_mul(v, s[1], Lf[2 * k + 1])
                    nc.vector.tensor_add(v, v, Lf[2 * k])
                    G.append(v)
                H = []
                for m in range(2):
                    _, v = tmp(f"hm{m}")
                    nc.vector.tensor_mul(v, s[2], G[2 * m + 1])
                    nc.vector.tensor_add(v, v, G[2 * m])
                    H.append(v)
                _, vl = tmp("lut")
                nc.vector.tensor_mul(vl, s[3], H[1])
                nc.vector.tensor_add(vl, vl, H[0])
                _, va = tmp("adds")
                nc.vector.tensor_add(va, s[0], s[1])
                nc.vector.tensor_add(va, va, s[2])
                _, vr = tmp("relu")
                nc.vector.tensor_scalar(vr, va, -2.0, 0.0, op0=ALU.add, op1=ALU.max)
                nc.vector.tensor_scalar(
                    vr, vr, 1.0, coef_sb[:, 16:17], op0=ALU.min, op1=ALU.mult
                )
                _, vq = tmp("step")
                nc.vector.tensor_scalar(
                    vq, va, 2.0, coef_sb[:, 17:18], op0=ALU.is_ge, op1=ALU.mult
                )
                # vo2 [128, (t k)] so partitions after transpose group by t
                vo2 = tpool.tile([128, TILES_GRP * T_IL], F32, tag="vo2")
                vo2v = vo2[:].rearrange("p (t k) -> p k t", t=T_IL, k=TILES_GRP)
                nc.vector.tensor_add(vo2v, vl, vr)
                nc.vector.tensor_add(vo2v, vo2v, vq)
                # two transposes -> voT[:, 128b:...] = [(tl k), p] for t = 4b+tl
                voT = ppvo.tile([128, 256], F32, tag="voT")
                nc.tensor.transpose(voT[:, 0:128], vo2[:, 0:128], ident[:])
                nc.tensor.transpose(voT[:, 128:256], vo2[:, 128:256], ident[:])
                for t in range(T_IL):
                    b, tl = divmod(t, 4)
                    nc.scalar.copy(
                        out=Sv[TILES_GRP * g : TILES_GRP * (g + 1), :, t],
                        in_=voT[32 * tl : 32 * (tl + 1), 128 * b : 128 * (b + 1)],
                    )
                nc.sync.dma_start(
                    out=outv[TILES_GRP * g : TILES_GRP * (g + 1)],
                    in_=S[TILES_GRP * g : TILES_GRP * (g + 1), :],
                )

            for g in range(n_grp):
                sn_sb = snpool.tile([128, 32 * TILES_GRP], F32, tag="snsb")
                for half in range(2):
                    sn_ps = ppsn.tile([128, 512], F32, tag="sn")
                    for kk in range(16):
                        k = 16 * half + kk
                        nd = g * TILES_GRP + k
                        xt = xpool.tile([128, 128 * 4], mm_dtype, tag="x")
                        nc.gpsimd.dma_start(out=xt[:], in_=x2v[nd])
                        xT_ps = ppxt.tile([128, 512], mm_dtype, tag="xT")
                        for h in range(4):
                            nc.tensor.transpose(
                                xT_ps[:, 128 * h : 128 * (h + 1)],
                                xt[:, 128 * h : 128 * (h + 1)],
                                identb[:],
                            )
                        xT_sb = xsbpool.tile([128, 512], mm_dtype, tag="xTsb")
                        if nd % 2 == 0:
                            nc.scalar.copy(out=xT_sb[:], in_=xT_ps[:])
                        else:
                            nc.vector.tensor_copy(out=xT_sb[:], in_=xT_ps[:])
                        for h in range(4):
                            nc.tensor.matmul(
                                sn_ps[:, 32 * kk : 32 * (kk + 1)],
                                lhsT=xT_sb[:, 128 * h : 128 * (h + 1)],
                                rhs=p8_sb[:, 32 * h : 32 * (h + 1)],
                                start=(h == 0),
                                stop=(h == 3),
                            )
                    if half == 0:
                        nc.vector.tensor_copy(out=sn_sb[:, 0:512], in_=sn_ps[:])
                    else:
                        nc.scalar.copy(out=sn_sb[:, 512:1024], in_=sn_ps[:])
                poly_group(sn_sb, g)
    nc.compile()
    return nc


def host_prep(choice_parameters, lut, lut_vs_add_choice_parameters, mm_np=ml_dtypes.bfloat16):
    cp = np.asarray(choice_parameters, dtype=np.float64)
    e = np.exp(cp - cp.max(axis=0, keepdims=True))
    probs = e / e.sum(axis=0, keepdims=True)  # [64,4]
    L = 1.0 / (1.0 + np.exp(-np.asarray(lut, dtype=np.float64)))
    m = np.asarray(lut_vs_add_choice_parameters, dtype=np.float64)
    em = np.exp(m - m.max())
    mix = em / em.sum()

    c = np.zeros(16)
    for Sm in range(16):
        v = L
        for ax in range(4):
            vec = np.array([1.0, -1.0]) if (Sm >> ax) & 1 else np.array([0.0, 1.0])
            v = np.tensordot(v, vec, axes=([0], [0]))
        c[Sm] = float(v) * mix[0]

    coef_row = np.zeros(18)
    for idx in range(8):
        coef_row[idx] = c[idx << 1]
        coef_row[8 + idx] = c[(idx << 1) | 1]
    coef_row[16] = mix[1]
    coef_row[17] = mix[2]
    coef = np.tile(coef_row.astype(np.float32)[None], (128, 1))

    # P8[h][ (t2,c), (j,t) ] = probs[c,j] * [t == 2h+t2]; packed as
    # p8[t2*64+c, 32*h + 8*j + t]
    p8 = np.zeros((128, 128), np.float64)
    for h in range(4):
        for t2 in range(2):
            for cc in range(64):
                for j in range(4):
                    p8[t2 * 64 + cc, 32 * h + 8 * j + (2 * h + t2)] = probs[cc, j]
    p8 = p8.astype(mm_np)
    return p8, coef


_NC_CACHE = {}


def _get_nc(R):
    if R not in _NC_CACHE:
        _NC_CACHE[R] = build_nc(R)
    return _NC_CACHE[R]


def run_on_hw(x, choice_parameters, lut, lut_vs_add_choice_parameters, **kw):
    x = np.ascontiguousarray(np.asarray(x, dtype=np.float32))
    R = x.shape[0] // N_CORES
    nc = _get_nc(R)
    p8, coef = host_prep(choice_parameters, lut, lut_vs_add_choice_parameters)
    in_maps = [
        {"x": np.ascontiguousarray(x[i * R : (i + 1) * R]), "p8": p8, "coef": coef}
        for i in range(N_CORES)
    ]
    res = run_bass_kernel_spmd(nc, in_maps, list(range(N_CORES)), **kw)
    out = np.concatenate([r["out"] for r in res.results], axis=0)
    return out, res


def kernel(x, choice_parameters, lut, lut_vs_add_choice_parameters):
    out, _ = run_on_hw(x, choice_parameters, lut, lut_vs_add_choice_parameters)
    return out


# revision 5
# speedup vs baseline: 4.1682x; 1.1450x over previous
# Trainium2 Bass kernel for nn_LAB_42906723287350.
#
#   probs = softmax(choice_parameters, axis=0); s = x @ probs
#   out = mix0*multilinear(sigmoid(lut); s) + mix1*clip(s0+s1+s2-2,0,1)
#         + mix2*(s0+s1+s2>=2)
#
# Data parallel over 8 cores (R=131072 rows each).  Row mapping per core:
#   row = ROWS_TILE*nd + T_IL*p + t   (nd = tile, p = partition, t in [0,T_IL))
# Per tile nd: SWDGE cast-DMA x [128, (t,c)] f32->bf16 (4KB/partition reads)
#   -> T_IL/2 PE transposes (bf16) -> xT [(t2,c), p] psum bf16 -> copy
#   -> T_IL/2 accumulating matmuls with xT as STATIONARY and P [128, 4*T_IL]
#      moving -> s in NATURAL layout [128 p, (j,t)] (no back-transpose).
# Per group g (TILES_GRP tiles): copy s psum->SBUF, 16-coeff Horner poly +
#   add path -> vo2 [128, (t,k)] -> PE transposes -> [(tl,k), p] psum
#   -> strided copies into S [(g,k), (p,t)] -> contiguous output DMA
#   (T_IL*512B/partition descriptors).
import numpy as np
import ml_dtypes

import concourse.bass as bass
import concourse.mybir as mybir
import concourse.tile as tile
from concourse import bacc
from concourse.bass_utils import run_bass_kernel_spmd
from concourse.masks import make_identity

N_CORES = 8
B_FULL = 1048576
CIN = 64
F32 = mybir.dt.float32
MM_DT = mybir.dt.bfloat16
ALU = mybir.AluOpType
AF = mybir.ActivationFunctionType

T_IL = 16                        # rows interleaved per partition
H_BLK = T_IL // 2                # 128-partition transpose sub-blocks per tile
ROWS_TILE = 128 * T_IL           # 2048 rows per x-tile
TILES_GRP = 32                   # tiles per poly group
ROWS_GRP = ROWS_TILE * TILES_GRP # 65536 rows
JT = 4 * T_IL                    # s-matmul output cols per tile (j,t)
TPB = 512 // JT                  # tiles per PSUM bank for s accumulation
TB = 128 // TILES_GRP            # t values per vo transpose block


def build_nc(R, mm_dtype=MM_DT):
    n_tiles = R // ROWS_TILE
    n_grp = n_tiles // TILES_GRP
    assert R == n_grp * ROWS_GRP and n_grp * TILES_GRP <= 128

    nc = bacc.Bacc()
    x_d = nc.dram_tensor("x", [R, CIN], F32, kind="ExternalInput")
    pm_d = nc.dram_tensor("pm", [128, H_BLK * JT], mm_dtype, kind="ExternalInput")
    coef_d = nc.dram_tensor("coef", [128, 18], F32, kind="ExternalInput")
    out_d = nc.dram_tensor("out", [R, 1], F32, kind="ExternalOutput")

    # x row = ROWS_TILE*nd + T_IL*p + t ; tile view [nd, p, (t c)]
    x2v = x_d[:].rearrange("(nd p t) c -> nd p (t c)", p=128, t=T_IL)
    # out row = ROWS_TILE*(g*TILES_GRP+k) + T_IL*p + t ; [(g k), (p t)]
    outv = out_d[:].rearrange(
        "(gk p t) one -> gk (p t one)", gk=n_grp * TILES_GRP, p=128, t=T_IL
    )

    with tile.TileContext(nc) as tc:
        with (
            tc.tile_pool(name="const", bufs=1) as cpool,
            tc.tile_pool(name="xin", bufs=6) as xpool,
            tc.tile_pool(name="xtsb", bufs=4) as xsbpool,
            tc.tile_pool(name="snat", bufs=2) as snpool,
            tc.tile_pool(name="tmp", bufs=2) as tpool,
            tc.tile_pool(name="outsb", bufs=1) as spool,
            tc.tile_pool(name="psxt", bufs=2, space="PSUM") as ppxt,
            tc.tile_pool(name="pssn", bufs=2, space="PSUM") as ppsn,
            tc.tile_pool(name="psvo", bufs=2, space="PSUM") as ppvo,
        ):
            identb = cpool.tile([128, 128], mm_dtype)
            make_identity(nc, identb[:])
            identf = cpool.tile([128, 128], F32)
            make_identity(nc, identf[:])
            pm_sb = cpool.tile([128, H_BLK * JT], mm_dtype)
            nc.sync.dma_start(out=pm_sb[:], in_=pm_d[:])
            coef_sb = cpool.tile([128, 18], F32)
            nc.sync.dma_start(out=coef_sb[:], in_=coef_d[:])

            S = spool.tile([n_grp * TILES_GRP, 128 * T_IL], F32)  # [(g k), (p t)]
            Sv = S[:].rearrange("q (p t) -> q p t", p=128, t=T_IL)

            def poly_group(sn_sb, g):
                # sn_sb [128, (k j t)] f32; s_j(row) at [p, k, j, t]
                sv = sn_sb[:].rearrange(
                    "p (k j t) -> p k j t", k=TILES_GRP, j=4, t=T_IL
                )
                s = [sv[:, :, jj, :] for jj in range(4)]

                def tmp(tag):
                    tl = tpool.tile([128, TILES_GRP * T_IL], F32, tag=tag)
                    return tl, tl[:].rearrange(
                        "p (k t) -> p k t", k=TILES_GRP, t=T_IL
                    )

                Lf = []
                for i in range(8):
                    _, v = tmp(f"leaf{i}")
                    nc.scalar.activation(
                        v, s[0], AF.Identity,
                        bias=coef_sb[:, i : i + 1],
                        scale=coef_sb[:, 8 + i : 9 + i],
                    )
                    Lf.append(v)
                G = []
                for k in range(4):
                    _, v = tmp(f"gm{k}")
                    nc.vector.tensor_mul(v, s[1], Lf[2 * k + 1])
                    nc.vector.tensor_add(v, v, Lf[2 * k])
                    G.append(v)
                H = []
                for m in range(2):
                    _, v = tmp(f"hm{m}")
                    nc.vector.tensor_mul(v, s[2], G[2 * m + 1])
                    nc.vector.tensor_add(v, v, G[2 * m])
                    H.append(v)
                _, vl = tmp("lut")
                nc.vector.tensor_mul(vl, s[3], H[1])
                nc.vector.tensor_add(vl, vl, H[0])
                _, va = tmp("adds")
                nc.vector.tensor_add(va, s[0], s[1])
                nc.vector.tensor_add(va, va, s[2])
                _, vr = tmp("relu")
                nc.vector.tensor_scalar(vr, va, -2.0, 0.0, op0=ALU.add, op1=ALU.max)
                nc.vector.tensor_scalar(
                    vr, vr, 1.0, coef_sb[:, 16:17], op0=ALU.min, op1=ALU.mult
                )
                _, vq = tmp("step")
                nc.vector.tensor_scalar(
                    vq, va, 2.0, coef_sb[:, 17:18], op0=ALU.is_ge, op1=ALU.mult
                )
                # vo2 [128, (t k)] so partitions after transpose group by t
                vo2 = tpool.tile([128, TILES_GRP * T_IL], F32, tag="vo2")
                vo2v = vo2[:].rearrange("p (t k) -> p k t", t=T_IL, k=TILES_GRP)
                nc.vector.tensor_add(vo2v, vl, vr)
                nc.vector.tensor_add(vo2v, vo2v, vq)
                # transposes: block b -> voT[:, 128b:...] = [(tl k), p], t = TB*b+tl
                n_blk = TILES_GRP * T_IL // 128
                voT = ppvo.tile([128, 128 * n_blk], F32, tag="voT")
                for b in range(n_blk):
                    nc.tensor.transpose(
                        voT[:, 128 * b : 128 * (b + 1)],
                        vo2[:, 128 * b : 128 * (b + 1)],
                        identf[:],
                    )
                for t in range(T_IL):
                    b, tl = divmod(t, TB)
                    nc.scalar.copy(
                        out=Sv[TILES_GRP * g : TILES_GRP * (g + 1), :, t],
                        in_=voT[
                            TILES_GRP * tl : TILES_GRP * (tl + 1),
                            128 * b : 128 * (b + 1),
                        ],
                    )
                nc.sync.dma_start(
                    out=outv[TILES_GRP * g : TILES_GRP * (g + 1)],
                    in_=S[TILES_GRP * g : TILES_GRP * (g + 1), :],
                )

            for g in range(n_grp):
                sn_sb = snpool.tile([128, JT * TILES_GRP], F32, tag="snsb")
                for part in range(TILES_GRP // TPB):
                    sn_ps = ppsn.tile([128, 512], F32, tag="sn")
                    for kk in range(TPB):
                        k = TPB * part + kk
                        nd = g * TILES_GRP + k
                        xt = xpool.tile([128, 128 * H_BLK], mm_dtype, tag="x")
                        nc.gpsimd.dma_start(out=xt[:], in_=x2v[nd])
                        xT_ps = ppxt.tile([128, 128 * H_BLK], mm_dtype, tag="xT")
                        for h in range(H_BLK):
                            nc.tensor.transpose(
                                xT_ps[:, 128 * h : 128 * (h + 1)],
                                xt[:, 128 * h : 128 * (h + 1)],
                                identb[:],
                            )
                        xT_sb = xsbpool.tile([128, 128 * H_BLK], mm_dtype, tag="xTsb")
                        if nd % 2 == 0:
                            nc.scalar.copy(out=xT_sb[:], in_=xT_ps[:])
                        else:
                            nc.vector.tensor_copy(out=xT_sb[:], in_=xT_ps[:])
                        for h in range(H_BLK):
                            nc.tensor.matmul(
                                sn_ps[:, JT * kk : JT * (kk + 1)],
                                lhsT=xT_sb[:, 128 * h : 128 * (h + 1)],
                                rhs=pm_sb[:, JT * h : JT * (h + 1)],
                                start=(h == 0),
                                stop=(h == H_BLK - 1),
                            )
                    if part % 2 == 0:
                        nc.vector.tensor_copy(
                            out=sn_sb[:, 512 * part : 512 * (part + 1)], in_=sn_ps[:]
                        )
                    else:
                        nc.scalar.copy(
                            out=sn_sb[:, 512 * part : 512 * (part + 1)], in_=sn_ps[:]
                        )
                poly_group(sn_sb, g)
    nc.compile()
    return nc


def host_prep(choice_parameters, lut, lut_vs_add_choice_parameters, mm_np=ml_dtypes.bfloat16):
    cp = np.asarray(choice_parameters, dtype=np.float64)
    e = np.exp(cp - cp.max(axis=0, keepdims=True))
    probs = e / e.sum(axis=0, keepdims=True)  # [64,4]
    L = 1.0 / (1.0 + np.exp(-np.asarray(lut, dtype=np.float64)))
    m = np.asarray(lut_vs_add_choice_parameters, dtype=np.float64)
    em = np.exp(m - m.max())
    mix = em / em.sum()

    c = np.zeros(16)
    for Sm in range(16):
        v = L
        for ax in range(4):
            vec = np.array([1.0, -1.0]) if (Sm >> ax) & 1 else np.array([0.0, 1.0])
            v = np.tensordot(v, vec, axes=([0], [0]))
        c[Sm] = float(v) * mix[0]

    coef_row = np.zeros(18)
    for idx in range(8):
        coef_row[idx] = c[idx << 1]
        coef_row[8 + idx] = c[(idx << 1) | 1]
    coef_row[16] = mix[1]
    coef_row[17] = mix[2]
    coef = np.tile(coef_row.astype(np.float32)[None], (128, 1))

    # pm[t2*64+c, JT*h + T_IL*j + t] = probs[c,j] * [t == 2h+t2]
    pm = np.zeros((128, H_BLK * JT), np.float64)
    for h in range(H_BLK):
        for t2 in range(2):
            for cc in range(64):
                for j in range(4):
                    pm[t2 * 64 + cc, JT * h + T_IL * j + (2 * h + t2)] = probs[cc, j]
    pm = pm.astype(mm_np)
    return pm, coef


_NC_CACHE = {}


def _get_nc(R):
    if R not in _NC_CACHE:
        _NC_CACHE[R] = build_nc(R)
    return _NC_CACHE[R]


def run_on_hw(x, choice_parameters, lut, lut_vs_add_choice_parameters, **kw):
    x = np.ascontiguousarray(np.asarray(x, dtype=np.float32))
    R = x.shape[0] // N_CORES
    nc = _get_nc(R)
    pm, coef = host_prep(choice_parameters, lut, lut_vs_add_choice_parameters)
    in_maps = [
        {"x": np.ascontiguousarray(x[i * R : (i + 1) * R]), "pm": pm, "coef": coef}
        for i in range(N_CORES)
    ]
    res = run_bass_kernel_spmd(nc, in_maps, list(range(N_CORES)), **kw)
    out = np.concatenate([r["out"] for r in res.results], axis=0)
    return out, res


def kernel(x, choice_parameters, lut, lut_vs_add_choice_parameters):
    out, _ = run_on_hw(x, choice_parameters, lut, lut_vs_add_choice_parameters)
    return out


# revision 7
# speedup vs baseline: 4.4967x; 1.0788x over previous
# Trainium2 Bass kernel for nn_LAB_42906723287350.
#
#   probs = softmax(choice_parameters, axis=0); s = x @ probs
#   out = mix0*multilinear(sigmoid(lut); s) + mix1*clip(s0+s1+s2-2,0,1)
#         + mix2*(s0+s1+s2>=2)
#
# Data parallel over 8 cores (R=131072 rows each).  Row mapping per core:
#   row = ROWS_TILE*nd + T_IL*p + t   (nd = tile, p = partition, t in [0,T_IL))
# Per tile nd: SWDGE cast-DMA x [128, (t,c)] f32->bf16 (4KB/partition reads)
#   -> T_IL/2 PE transposes (bf16) -> xT [(t2,c), p] psum bf16 -> copy
#   -> T_IL/2 accumulating matmuls with xT as STATIONARY and P [128, 4*T_IL]
#      moving -> s in NATURAL layout [128 p, (j,t)] (no back-transpose).
# Per group g (TILES_GRP tiles): copy s psum->SBUF, 16-coeff Horner poly +
#   add path -> vo2 [128, (t,k)] -> PE transposes -> [(tl,k), p] psum
#   -> strided copies into S [(g,k), (p,t)] -> contiguous output DMA
#   (T_IL*512B/partition descriptors).
import numpy as np
import ml_dtypes

import concourse.bass as bass
import concourse.mybir as mybir
import concourse.tile as tile
from concourse import bacc
from concourse.bass_utils import run_bass_kernel_spmd
from concourse.masks import make_identity

N_CORES = 8
B_FULL = 1048576
CIN = 64
F32 = mybir.dt.float32
MM_DT = mybir.dt.bfloat16
ALU = mybir.AluOpType
AF = mybir.ActivationFunctionType

T_IL = 16                        # rows interleaved per partition
H_BLK = T_IL // 2                # 128-partition transpose sub-blocks per tile
ROWS_TILE = 128 * T_IL           # 2048 rows per x-tile
TILES_GRP = 32                   # tiles per poly group
ROWS_GRP = ROWS_TILE * TILES_GRP # 65536 rows
JT = 4 * T_IL                    # s-matmul output cols per tile (j,t)
TPB = 512 // JT                  # tiles per PSUM bank for s accumulation
TB = 128 // TILES_GRP            # t values per vo transpose block


def build_nc(R, mm_dtype=MM_DT):
    n_tiles = R // ROWS_TILE
    n_grp = n_tiles // TILES_GRP
    assert R == n_grp * ROWS_GRP and n_grp * TILES_GRP <= 128

    nc = bacc.Bacc()
    x_d = nc.dram_tensor("x", [R, CIN], F32, kind="ExternalInput")
    pm_d = nc.dram_tensor("pm", [128, H_BLK * JT], mm_dtype, kind="ExternalInput")
    coef_d = nc.dram_tensor("coef", [128, 18], F32, kind="ExternalInput")
    out_d = nc.dram_tensor("out", [R, 1], F32, kind="ExternalOutput")

    # x row = ROWS_TILE*nd + T_IL*p + t ; tile view [nd, p, (t c)]
    x2v = x_d[:].rearrange("(nd p t) c -> nd p (t c)", p=128, t=T_IL)
    # out row = ROWS_TILE*(g*TILES_GRP+k) + T_IL*p + t ; [(g k), (p t)]
    outv = out_d[:].rearrange(
        "(gk p t) one -> gk (p t one)", gk=n_grp * TILES_GRP, p=128, t=T_IL
    )

    with tile.TileContext(nc) as tc:
        with (
            tc.tile_pool(name="const", bufs=1) as cpool,
            tc.tile_pool(name="xin", bufs=8) as xpool,
            tc.tile_pool(name="xtsb", bufs=6) as xsbpool,
            tc.tile_pool(name="tmp", bufs=2) as tpool,
            tc.tile_pool(name="outsb", bufs=1) as spool,
            tc.tile_pool(name="psxt", bufs=2, space="PSUM") as ppxt,
            tc.tile_pool(name="pssn", bufs=2, space="PSUM") as ppsn,
            tc.tile_pool(name="psvo", bufs=2, space="PSUM") as ppvo,
        ):
            identb = cpool.tile([128, 128], mm_dtype)
            make_identity(nc, identb[:])
            identf = cpool.tile([128, 128], F32)
            make_identity(nc, identf[:])
            pm_sb = cpool.tile([128, H_BLK * JT], mm_dtype)
            nc.sync.dma_start(out=pm_sb[:], in_=pm_d[:])
            coef_sb = cpool.tile([128, 18], F32)
            nc.sync.dma_start(out=coef_sb[:], in_=coef_d[:])

            S = spool.tile([n_grp * TILES_GRP, 128 * T_IL], F32)  # [(g k), (p t)]
            Sv = S[:].rearrange("q (p t) -> q p t", p=128, t=T_IL)

            KH = TILES_GRP // 2   # tiles per poly half

            def poly_half(sn_ps, vo2, half):
                # sn_ps [128, (k j t)] f32 (PSUM); s_j(row) at [p, k, j, t]
                sv = sn_ps[:].rearrange(
                    "p (k j t) -> p k j t", k=KH, j=4, t=T_IL
                )
                s = [sv[:, :, jj, :] for jj in range(4)]

                def tmp(tag):
                    tl = tpool.tile([128, KH * T_IL], F32, tag=tag)
                    return tl, tl[:].rearrange(
                        "p (k t) -> p k t", k=KH, t=T_IL
                    )

                Lf = []
                for i in range(8):
                    _, v = tmp(f"leaf{i}")
                    nc.scalar.activation(
                        v, s[0], AF.Identity,
                        bias=coef_sb[:, i : i + 1],
                        scale=coef_sb[:, 8 + i : 9 + i],
                    )
                    Lf.append(v)
                G = []
                for k in range(4):
                    _, v = tmp(f"gm{k}")
                    nc.vector.tensor_mul(v, s[1], Lf[2 * k + 1])
                    nc.vector.tensor_add(v, v, Lf[2 * k])
                    G.append(v)
                H = []
                for m in range(2):
                    _, v = tmp(f"hm{m}")
                    nc.vector.tensor_mul(v, s[2], G[2 * m + 1])
                    nc.vector.tensor_add(v, v, G[2 * m])
                    H.append(v)
                _, vl = tmp("lut")
                nc.vector.tensor_mul(vl, s[3], H[1])
                nc.vector.tensor_add(vl, vl, H[0])
                _, va0 = tmp("adds0")
                nc.vector.tensor_copy(out=va0, in_=s[0])
                _, va = tmp("adds")
                nc.vector.tensor_add(va, va0, s[1])
                nc.vector.tensor_add(va, va, s[2])
                _, vr = tmp("relu")
                nc.vector.tensor_scalar(vr, va, -2.0, 0.0, op0=ALU.add, op1=ALU.max)
                nc.vector.tensor_scalar(
                    vr, vr, 1.0, coef_sb[:, 16:17], op0=ALU.min, op1=ALU.mult
                )
                _, vq = tmp("step")
                nc.vector.tensor_scalar(
                    vq, va, 2.0, coef_sb[:, 17:18], op0=ALU.is_ge, op1=ALU.mult
                )
                # write into this half's k-slice of the group vo2 [128, (t k)]
                vo2v = vo2[:].rearrange("p (t k) -> p k t", t=T_IL, k=TILES_GRP)[
                    :, KH * half : KH * (half + 1), :
                ]
                nc.vector.tensor_add(vo2v, vl, vr)
                nc.vector.tensor_add(vo2v, vo2v, vq)

            def store_group(vo2, g):
                # transposes: block b -> voT[:, 128b:...] = [(tl k), p], t = TB*b+tl
                n_blk = TILES_GRP * T_IL // 128
                voT = ppvo.tile([128, 128 * n_blk], F32, tag="voT")
                for b in range(n_blk):
                    nc.tensor.transpose(
                        voT[:, 128 * b : 128 * (b + 1)],
                        vo2[:, 128 * b : 128 * (b + 1)],
                        identf[:],
                    )
                for t in range(T_IL):
                    b, tl = divmod(t, TB)
                    dst = Sv[TILES_GRP * g : TILES_GRP * (g + 1), :, t]
                    srcv = voT[
                        TILES_GRP * tl : TILES_GRP * (tl + 1),
                        128 * b : 128 * (b + 1),
                    ]
                    if t % 2 == 0:
                        nc.scalar.copy(out=dst, in_=srcv)
                    else:
                        nc.vector.tensor_copy(out=dst, in_=srcv)
                nc.sync.dma_start(
                    out=outv[TILES_GRP * g : TILES_GRP * (g + 1)],
                    in_=S[TILES_GRP * g : TILES_GRP * (g + 1), :],
                )

            for g in range(n_grp):
                vo2 = tpool.tile([128, TILES_GRP * T_IL], F32, tag="vo2")
                for half in range(2):
                    sn_ps = ppsn.tile([128, JT * (TILES_GRP // 2)], F32, tag="sn")
                    for kk in range(TILES_GRP // 2):
                        k = (TILES_GRP // 2) * half + kk
                        nd = g * TILES_GRP + k
                        xt = xpool.tile([128, 128 * H_BLK], mm_dtype, tag="x")
                        nc.gpsimd.dma_start(out=xt[:], in_=x2v[nd])
                        xT_ps = ppxt.tile([128, 128 * H_BLK], mm_dtype, tag="xT")
                        for h in range(H_BLK):
                            nc.tensor.transpose(
                                xT_ps[:, 128 * h : 128 * (h + 1)],
                                xt[:, 128 * h : 128 * (h + 1)],
                                identb[:],
                            )
                        xT_sb = xsbpool.tile([128, 128 * H_BLK], mm_dtype, tag="xTsb")
                        if nd % 2 == 0:
                            nc.scalar.copy(out=xT_sb[:], in_=xT_ps[:])
                        else:
                            nc.vector.tensor_copy(out=xT_sb[:], in_=xT_ps[:])
                        for h in range(H_BLK):
                            nc.tensor.matmul(
                                sn_ps[:, JT * kk : JT * (kk + 1)],
                                lhsT=xT_sb[:, 128 * h : 128 * (h + 1)],
                                rhs=pm_sb[:, JT * h : JT * (h + 1)],
                                start=(h == 0),
                                stop=(h == H_BLK - 1),
                            )
                    poly_half(sn_ps, vo2, half)
                store_group(vo2, g)
    nc.compile()
    return nc


def host_prep(choice_parameters, lut, lut_vs_add_choice_parameters, mm_np=ml_dtypes.bfloat16):
    cp = np.asarray(choice_parameters, dtype=np.float64)
    e = np.exp(cp - cp.max(axis=0, keepdims=True))
    probs = e / e.sum(axis=0, keepdims=True)  # [64,4]
    L = 1.0 / (1.0 + np.exp(-np.asarray(lut, dtype=np.float64)))
    m = np.asarray(lut_vs_add_choice_parameters, dtype=np.float64)
    em = np.exp(m - m.max())
    mix = em / em.sum()

    c = np.zeros(16)
    for Sm in range(16):
        v = L
        for ax in range(4):
            vec = np.array([1.0, -1.0]) if (Sm >> ax) & 1 else np.array([0.0, 1.0])
            v = np.tensordot(v, vec, axes=([0], [0]))
        c[Sm] = float(v) * mix[0]

    coef_row = np.zeros(18)
    for idx in range(8):
        coef_row[idx] = c[idx << 1]
        coef_row[8 + idx] = c[(idx << 1) | 1]
    coef_row[16] = mix[1]
    coef_row[17] = mix[2]
    coef = np.tile(coef_row.astype(np.float32)[None], (128, 1))

    # pm[t2*64+c, JT*h + T_IL*j + t] = probs[c,j] * [t == 2h+t2]
    pm = np.zeros((128, H_BLK * JT), np.float64)
    for h in range(H_BLK):
        for t2 in range(2):
            for cc in range(64):
                for j in range(4):
                    pm[t2 * 64 + cc, JT * h + T_IL * j + (2 * h + t2)] = probs[cc, j]
    pm = pm.astype(mm_np)
    return pm, coef


_NC_CACHE = {}


def _get_nc(R):
    if R not in _NC_CACHE:
        _NC_CACHE[R] = build_nc(R)
    return _NC_CACHE[R]


def run_on_hw(x, choice_parameters, lut, lut_vs_add_choice_parameters, **kw):
    x = np.ascontiguousarray(np.asarray(x, dtype=np.float32))
    R = x.shape[0] // N_CORES
    nc = _get_nc(R)
    pm, coef = host_prep(choice_parameters, lut, lut_vs_add_choice_parameters)
    in_maps = [
        {"x": np.ascontiguousarray(x[i * R : (i + 1) * R]), "pm": pm, "coef": coef}
        for i in range(N_CORES)
    ]
    res = run_bass_kernel_spmd(nc, in_maps, list(range(N_CORES)), **kw)
    out = np.concatenate([r["out"] for r in res.results], axis=0)
    return out, res


def kernel(x, choice_parameters, lut, lut_vs_add_choice_parameters):
    out, _ = run_on_hw(x, choice_parameters, lut, lut_vs_add_choice_parameters)
    return out
